# revision 11
# baseline (speedup 1.0000x reference)
"""DenseGrid multi-resolution 1-D linear interpolation on 8 Trainium2 cores.

Math: out[n, l, f] = (1-fr)*storage[off_l + i0, f] + fr*storage[off_l + i0 + 1, f]
with i0 = floor(x[n]*(R_l-1)), fr = frac(x[n]*(R_l-1)).

Device algorithm (per core, data-parallel over N):
  The whole lookup+lerp is one matmul against "tent" (hat) basis values:
      out[ch=(l,f), n] = sum_{l,j} tent(m_l*x_n - j) * storage[off_l + j, f]
  where tent(v) = relu(1 - |v|) and m_l = R_l - 1. The 320 (l,j) rows are
  split into K-chunks: k=0 rows 0..127, k=1 rows 128..255, k=2 rows 256..319
  (64 rows; two chunks' worth are packed into one 128-partition tile).

  1. PE:  psA[(l,j)-row, n] = m_l*(xh_n + xl_n) - j*1  (K=3 fp16 matmuls with
          a ones row; xh/xl is a lossless hi/lo split of fp32 x so psA is
          exact to ~2^-23; k=0/1/2a/2b run concurrently via PE row/col tiling)
  2. DVE: T = relu(1 - |psA|) for k=0,1 (fused custom DVE op, fp16 out)
     ACT: same for the packed k=2 pair tile (Abs pass then Relu pass)
  3. PE:  psO[n-part, ch] = sum_k T_k.T @ table_k     (PSUM accumulation)
  4. ACT: psO -> SBUF, DMA out (n-major rows, contiguous per partition)
Tables are host-side layout prep of the tiny (320x4) storage tensor,
replicated to all cores (data-parallel sharding over points).
"""

import numpy as np

import concourse.bacc as bacc
import concourse.mybir as mybir
import concourse.tile as tile
from concourse.bass_utils import run_bass_kernel_spmd

# ----------------------------------------------------------------------------
# Problem constants (hardcoded per spec)
# ----------------------------------------------------------------------------
N_FULL = 1_048_576
LEVELS = 16
FEAT = 4
N_CORES = 8
NCP = N_FULL // N_CORES            # points per core = 131072
P = 128                            # SBUF partitions
IP = NCP // P                      # i-slots per partition = 1024
RESOLUTIONS = [2 * i + 1 for i in range(2, LEVELS + 2)]   # [5,7,...,35]
KROWS = sum(RESOLUTIONS)           # 320 tent rows
K2 = KROWS - 2 * P                 # rows in the third (partial) chunk = 64

CHUNK = 1024                       # points per inner chunk (fp16 moving max)
GI = CHUNK // P                    # 128-pt groups per chunk = 8
SUPER_I = 64                       # i-slots per super-chunk (output DMA batch)
DVE_COPY = 0                       # copy columns handled by DVE (rest on ACT)

# ----------------------------------------------------------------------------
# Custom DVE op: tent(v) = relu(1 - |v|)
# ----------------------------------------------------------------------------
_TENT_NAME = "TENT0_ANT_DG"


def _register_tent_op():
    from concourse import dve_ops
    from concourse.dve_spec import Spec, Src0, One, Zero, relu, maxx, lower
    from concourse.dve_table_gen import DveOpSpec

    if any(op.name == _TENT_NAME for op in dve_ops.OPS):
        return next(op for op in dve_ops.OPS if op.name == _TENT_NAME)

    body = relu(One - maxx(Src0, Zero - Src0))
    spec = Spec(
        body=body,
        reference=lambda in0, in1, s0, s1, imm2: np.maximum(
            1.0 - np.abs(np.asarray(in0, np.float32)), 0.0
        ),
    )
    shas = {}
    for ver in ("v3", "v4"):
        s = DveOpSpec(name=_TENT_NAME, opcode=0, uops=lower(spec, ver=ver), rd1_en=False)
        shas[ver] = s.sha(ver)
    op = dve_ops.DveOp(_TENT_NAME, spec, subdim=False, uops_sha=shas)
    dve_ops.OPS.append(op)
    dve_ops._SUB_OPCODE_FOR_NAME[op.name] = (
        dve_ops._CUSTOM_DVE_ROW_BASE + len(dve_ops.OPS) - 1
    )
    dve_ops.CUSTOM_DVE_SPECS[op.name] = op.spec
    return op


# ----------------------------------------------------------------------------
# Host table prep (tiny: 320x4 -> packed SBUF layouts; pure layout/dtype work)
# ----------------------------------------------------------------------------
def make_tables(storage, resolutions):
    storage = np.asarray(storage, np.float32)
    res = np.asarray(resolutions, np.int64)
    offs = np.concatenate([[0], np.cumsum(res)[:-1]])
    row_m = np.zeros(KROWS, np.float32)
    row_j = np.zeros(KROWS, np.float32)
    mvals = np.zeros((KROWS, FEAT * LEVELS), np.float32)   # [krow, ch]
    r = 0
    for l in range(LEVELS):
        m = int(res[l]) - 1
        for j in range(int(res[l])):
            row_m[r] = m
            row_j[r] = j
            mvals[r, 4 * l : 4 * l + 4] = storage[offs[l] + j]
            r += 1
    assert r == KROWS

    # affine stationaries (m, m, -j): k=0 at rows 0-2 cols 0:128, k=1 at rows
    # 32-34 cols 0:128, k=2 copy A at rows 64-66 cols 0:64 (chunk 2t) and
    # copy B at rows 96-98 cols 64:128 (chunk 2t+1)
    mstat = np.zeros((P, P), np.float16)
    for k, (rbase, cbase, rows) in enumerate(
        [(0, 0, slice(0, P)), (32, 0, slice(P, 2 * P)),
         (64, 0, slice(2 * P, KROWS)), (96, 64, slice(2 * P, KROWS))]
    ):
        n = rows.stop - rows.start
        mstat[rbase, cbase : cbase + n] = row_m[rows]
        mstat[rbase + 1, cbase : cbase + n] = row_m[rows]
        mstat[rbase + 2, cbase : cbase + n] = -row_j[rows]

    # value table: cols k*64+ch for k=0,1 on all 128 partitions; k=2 values
    # on partitions 0:64 (cols 128:192) and replicated on partitions 64:128
    # (cols 192:256) for the packed pair tile's upper half
    mv = np.zeros((P, 4 * 64), np.float16)
    mv[:, 0:64] = mvals[0:P].astype(np.float16)
    mv[:, 64:128] = mvals[P : 2 * P].astype(np.float16)
    mv[0:K2, 128:192] = mvals[2 * P : KROWS].astype(np.float16)
    mv[64 : 64 + K2, 192:256] = mvals[2 * P : KROWS].astype(np.float16)
    return mstat, mv


# ----------------------------------------------------------------------------
# Bass program (SPMD, one program for all cores)
# ----------------------------------------------------------------------------
def build_program(ncp=NCP):
    tent_op = _register_tent_op()
    ip = ncp // P                       # i-slots
    n_super = max(1, ip // SUPER_I)
    super_i = ip // n_super             # i-slots per super-chunk
    chunks_per_super = super_i // GI
    assert chunks_per_super % 2 == 0 or chunks_per_super == 1
    sup_pts = super_i * P               # points per super-chunk

    f32 = mybir.dt.float32
    f16 = mybir.dt.float16
    AF = mybir.ActivationFunctionType

    nc = bacc.Bacc()
    x_ext = nc.declare_dram_parameter("x", [3, ncp], f16, isOutput=False)
    mstat_ext = nc.declare_dram_parameter("mstat", [P, P], f16, isOutput=False)
    mv_ext = nc.declare_dram_parameter("mv", [P, 4 * 64], f16, isOutput=False)
    out_ext = nc.declare_dram_parameter("out", [P, ip, 64], f32, isOutput=True)

    with tile.TileContext(nc) as tc:
        with (
            tc.tile_pool(name="consts", bufs=1) as cpool,
            tc.tile_pool(name="xin", bufs=2) as xpool,
            tc.tile_pool(name="tent", bufs=3) as tpool,
            tc.tile_pool(name="obuf", bufs=2) as opool,
            tc.tile_pool(name="psA", bufs=1, space="PSUM") as psa_pool,
            tc.tile_pool(name="psO", bufs=2, space="PSUM") as pso_pool,
        ):
            mstat_t = cpool.tile([P, P], f16, tag="mstat")
            mv_t = cpool.tile([P, 4 * 64], f16, tag="mv")
            nc.sync.dma_start(out=mstat_t[:], in_=mstat_ext[:])
            nc.sync.dma_start(out=mv_t[:], in_=mv_ext[:])

            for s in range(n_super):
                # x rows (xh, xl, ones) replicated at partitions 0,32,64,96
                x_t = xpool.tile([99, sup_pts], f16, tag="x", name=f"x_{s}")
                for rb in (0, 32, 64, 96):
                    nc.sync.dma_start(
                        out=x_t[rb : rb + 3, :],
                        in_=x_ext[:, s * sup_pts : (s + 1) * sup_pts],
                    )
                o_t = opool.tile([P, super_i * 64], f32, tag="o", name=f"o_{s}")
                npair = max(1, chunks_per_super // 2)
                for pr in range(npair):
                    # ---- packed k=2 affine for the chunk pair ----
                    psA2 = psa_pool.tile([P, CHUNK], f32, tag="A2", name=f"psA2_{s}_{pr}")
                    T2 = tpool.tile([P, CHUNK], f16, tag="T2", name=f"T2_{s}_{pr}")
                    for half_ch in range(min(2, chunks_per_super)):
                        cl = 2 * pr + half_ch
                        rb, cb = (64, 0) if half_ch == 0 else (96, 64)
                        for h in range(2):  # fp32 PSUM: 512 cols per matmul
                            xs = slice(cl * CHUNK + h * 512, cl * CHUNK + (h + 1) * 512)
                            nc.tensor.matmul(
                                psA2[cb : cb + 64, h * 512 : (h + 1) * 512],
                                lhsT=mstat_t[rb : rb + 3, cb : cb + 64],
                                rhs=x_t[rb : rb + 3, xs],
                                start=True,
                                stop=True,
                                tile_position=(rb, cb),
                            )
                    nc.scalar.activation(T2[:], psA2[:], AF.Abs)
                    nc.scalar.activation(T2[:], T2[:], AF.Relu, bias=1.0, scale=-1.0)

                    for half_ch in range(min(2, chunks_per_super)):
                        cl = 2 * pr + half_ch
                        t2base = 0 if half_ch == 0 else 64
                        mvc2 = 128 if half_ch == 0 else 192
                        psA = [
                            psa_pool.tile([P, CHUNK], f32, tag=f"A{k}", name=f"psA{k}_{s}_{cl}")
                            for k in range(2)
                        ]
                        T = [
                            tpool.tile([P, CHUNK], f16, tag=f"T{k}", name=f"T{k}_{s}_{cl}")
                            for k in range(2)
                        ]
                        for k in range(2):
                            for h in range(2):
                                xs = slice(cl * CHUNK + h * 512, cl * CHUNK + (h + 1) * 512)
                                nc.tensor.matmul(
                                    psA[k][:, h * 512 : (h + 1) * 512],
                                    lhsT=mstat_t[32 * k : 32 * k + 3, :],
                                    rhs=x_t[32 * k : 32 * k + 3, xs],
                                    start=True,
                                    stop=True,
                                    tile_position=(32 * k, 0),
                                )
                            nc.vector._custom_dve(tent_op, out=T[k][:], in0=psA[k][:])

                        psO = pso_pool.tile([P, GI * 64], f32, tag="O", name=f"psO_{s}_{cl}")
                        for g in range(GI):
                            nc.tensor.matmul(
                                psO[:, g * 64 : (g + 1) * 64],
                                lhsT=T[0][:, g * P : (g + 1) * P],
                                rhs=mv_t[:, 0:64],
                                start=True,
                                stop=False,
                            )
                            nc.tensor.matmul(
                                psO[:, g * 64 : (g + 1) * 64],
                                lhsT=T[1][:, g * P : (g + 1) * P],
                                rhs=mv_t[:, 64:128],
                                start=False,
                                stop=False,
                            )
                            nc.tensor.matmul(
                                psO[:, g * 64 : (g + 1) * 64],
                                lhsT=T2[t2base : t2base + 64, g * P : (g + 1) * P],
                                rhs=mv_t[t2base : t2base + 64, mvc2 : mvc2 + 64],
                                start=False,
                                stop=True,
                            )
                        oc = cl * GI * 64
                        if DVE_COPY:
                            nc.vector.tensor_copy(
                                o_t[:, oc : oc + DVE_COPY], psO[:, 0:DVE_COPY]
                            )
                        nc.scalar.copy(
                            o_t[:, oc + DVE_COPY : oc + GI * 64], psO[:, DVE_COPY:]
                        )
                nc.sync.dma_start(
                    out=out_ext[:, s * super_i : (s + 1) * super_i, :],
                    in_=o_t[:],
                )
    nc.finalize()
    return nc


# ----------------------------------------------------------------------------
# Host entry point
# ----------------------------------------------------------------------------
def _proc_order(x_shard):
    """Permute points into the device processing order n' = c*CHUNK + g*128 + q
    (point = q*IP + c*GI + g), then split fp32 x losslessly into an fp16
    (hi, lo) pair for the PE's fp16 datapath. Pure layout/precision prep."""
    ncp = x_shard.shape[0]
    ip = ncp // P
    xp = np.ascontiguousarray(
        x_shard.reshape(P, ip // GI, GI).transpose(1, 2, 0)
    ).reshape(-1)
    xh = xp.astype(np.float16)
    xl = (xp - xh.astype(np.float32)).astype(np.float16)
    ones = np.ones_like(xh)
    return np.stack([xh, xl, ones])


_PROGRAM_CACHE = {}


def kernel(x, storage, resolutions):
    x = np.asarray(x, np.float32).reshape(-1)
    assert x.shape[0] == N_FULL
    mstat, mv = make_tables(storage, resolutions)

    if NCP not in _PROGRAM_CACHE:
        _PROGRAM_CACHE[NCP] = build_program(NCP)
    nc = _PROGRAM_CACHE[NCP]

    in_maps = []
    for c in range(N_CORES):
        shard = x[c * NCP : (c + 1) * NCP]
        in_maps.append({"x": _proc_order(shard), "mstat": mstat, "mv": mv})
    res = run_bass_kernel_spmd(nc, in_maps, list(range(N_CORES)))
    outs = [r["out"].reshape(NCP, LEVELS, FEAT) for r in res.results]
    return np.concatenate(outs, axis=0)


# revision 12
# speedup vs baseline: 1.0510x; 1.0510x over previous
"""DenseGrid multi-resolution 1-D linear interpolation on 8 Trainium2 cores.

Math: out[n, l, f] = (1-fr)*storage[off_l + i0, f] + fr*storage[off_l + i0 + 1, f]
with i0 = floor(x[n]*(R_l-1)), fr = frac(x[n]*(R_l-1)).

Device algorithm (per core, data-parallel over N):
  The whole lookup+lerp is one matmul against "tent" (hat) basis values:
      out[ch=(l,f), n] = sum_{l,j} tent(m_l*x_n - j) * storage[off_l + j, f]
  where tent(v) = relu(1 - |v|) and m_l = R_l - 1. The 320 (l,j) rows are
  split into K-chunks: k=0 rows 0..127, k=1 rows 128..255, k=2 rows 256..319
  (64 rows; two 512-pt chunks' worth are packed into one 128-partition tile).

  1. PE:  psA[(l,j)-row, n] = m_l*(xh_n + xl_n) - j*1  (K=3 fp16 matmuls with
          a ones row; xh/xl is a lossless hi/lo split of fp32 x so psA is
          exact to ~2^-23; k=0/1/2a/2b run concurrently via PE row/col tiling)
  2. DVE: T = relu(1 - |psA|) for k=0,1 (fused custom DVE op, fp16 out)
     ACT: same for the packed k=2 pair tile (Abs pass then Relu pass)
  3. PE:  psO[n-part, ch] = sum_k T_k.T @ table_k     (PSUM accumulation)
  4. ACT: psO -> SBUF, DMA out (n-major rows, contiguous per partition)
  The emission is software-pipelined pair-by-pair (front of pair p+1 is
  emitted before the mains of pair p) so the PE never starves on tents.
Tables are host-side layout prep of the tiny (320x4) storage tensor,
replicated to all cores (data-parallel sharding over points).
"""

import numpy as np

import concourse.bacc as bacc
import concourse.mybir as mybir
import concourse.tile as tile
from concourse.bass_utils import run_bass_kernel_spmd

# ----------------------------------------------------------------------------
# Problem constants (hardcoded per spec)
# ----------------------------------------------------------------------------
N_FULL = 1_048_576
LEVELS = 16
FEAT = 4
N_CORES = 8
NCP = N_FULL // N_CORES            # points per core = 131072
P = 128                            # SBUF partitions
IP = NCP // P                      # i-slots per partition = 1024
RESOLUTIONS = [2 * i + 1 for i in range(2, LEVELS + 2)]   # [5,7,...,35]
KROWS = sum(RESOLUTIONS)           # 320 tent rows
K2 = KROWS - 2 * P                 # rows in the third (partial) chunk = 64

CHUNK = 512                        # points per chunk
GI = CHUNK // P                    # 128-pt groups per chunk = 4
PAIR = 2 * CHUNK                   # points per software-pipeline stage
SUPER_I = 64                       # i-slots per super-chunk (output DMA batch)

# ----------------------------------------------------------------------------
# Custom DVE op: tent(v) = relu(1 - |v|)
# ----------------------------------------------------------------------------
_TENT_NAME = "TENT0_ANT_DG"


def _register_tent_op():
    from concourse import dve_ops
    from concourse.dve_spec import Spec, Src0, One, Zero, relu, maxx, lower
    from concourse.dve_table_gen import DveOpSpec

    if any(op.name == _TENT_NAME for op in dve_ops.OPS):
        return next(op for op in dve_ops.OPS if op.name == _TENT_NAME)

    body = relu(One - maxx(Src0, Zero - Src0))
    spec = Spec(
        body=body,
        reference=lambda in0, in1, s0, s1, imm2: np.maximum(
            1.0 - np.abs(np.asarray(in0, np.float32)), 0.0
        ),
    )
    shas = {}
    for ver in ("v3", "v4"):
        s = DveOpSpec(name=_TENT_NAME, opcode=0, uops=lower(spec, ver=ver), rd1_en=False)
        shas[ver] = s.sha(ver)
    op = dve_ops.DveOp(_TENT_NAME, spec, subdim=False, uops_sha=shas)
    dve_ops.OPS.append(op)
    dve_ops._SUB_OPCODE_FOR_NAME[op.name] = (
        dve_ops._CUSTOM_DVE_ROW_BASE + len(dve_ops.OPS) - 1
    )
    dve_ops.CUSTOM_DVE_SPECS[op.name] = op.spec
    return op


# ----------------------------------------------------------------------------
# Host table prep (tiny: 320x4 -> packed SBUF layouts; pure layout/dtype work)
# ----------------------------------------------------------------------------
def make_tables(storage, resolutions):
    storage = np.asarray(storage, np.float32)
    res = np.asarray(resolutions, np.int64)
    offs = np.concatenate([[0], np.cumsum(res)[:-1]])
    row_m = np.zeros(KROWS, np.float32)
    row_j = np.zeros(KROWS, np.float32)
    mvals = np.zeros((KROWS, FEAT * LEVELS), np.float32)   # [krow, ch]
    r = 0
    for l in range(LEVELS):
        m = int(res[l]) - 1
        for j in range(int(res[l])):
            row_m[r] = m
            row_j[r] = j
            mvals[r, 4 * l : 4 * l + 4] = storage[offs[l] + j]
            r += 1
    assert r == KROWS

    # affine stationaries (m, m, -j): k=0 at rows 0-2 cols 0:128, k=1 at rows
    # 32-34 cols 0:128, k=2 copy A at rows 64-66 cols 0:64 (even chunk) and
    # copy B at rows 96-98 cols 64:128 (odd chunk)
    mstat = np.zeros((P, P), np.float16)
    for rbase, cbase, rows in [
        (0, 0, slice(0, P)),
        (32, 0, slice(P, 2 * P)),
        (64, 0, slice(2 * P, KROWS)),
        (96, 64, slice(2 * P, KROWS)),
    ]:
        n = rows.stop - rows.start
        mstat[rbase, cbase : cbase + n] = row_m[rows]
        mstat[rbase + 1, cbase : cbase + n] = row_m[rows]
        mstat[rbase + 2, cbase : cbase + n] = -row_j[rows]

    # value table: cols k*64+ch for k=0,1 on all 128 partitions; k=2 values
    # on partitions 0:64 (cols 128:192) and replicated on partitions 64:128
    # (cols 192:256) for the packed pair tile's upper half
    mv = np.zeros((P, 4 * 64), np.float16)
    mv[:, 0:64] = mvals[0:P].astype(np.float16)
    mv[:, 64:128] = mvals[P : 2 * P].astype(np.float16)
    mv[0:K2, 128:192] = mvals[2 * P : KROWS].astype(np.float16)
    mv[64 : 64 + K2, 192:256] = mvals[2 * P : KROWS].astype(np.float16)
    return mstat, mv


# ----------------------------------------------------------------------------
# Bass program (SPMD, one program for all cores)
# ----------------------------------------------------------------------------
def build_program(ncp=NCP):
    tent_op = _register_tent_op()
    ip = ncp // P                        # i-slots
    n_super = max(1, ip // SUPER_I)
    super_i = ip // n_super              # i-slots per super-chunk
    sup_pts = super_i * P
    pairs_per_super = sup_pts // PAIR
    n_pairs = n_super * pairs_per_super

    f32 = mybir.dt.float32
    f16 = mybir.dt.float16
    AF = mybir.ActivationFunctionType

    nc = bacc.Bacc()
    x_ext = nc.declare_dram_parameter("x", [3, ncp], f16, isOutput=False)
    mstat_ext = nc.declare_dram_parameter("mstat", [P, P], f16, isOutput=False)
    mv_ext = nc.declare_dram_parameter("mv", [P, 4 * 64], f16, isOutput=False)
    out_ext = nc.declare_dram_parameter("out", [P, ip, 64], f32, isOutput=True)

    with tile.TileContext(nc) as tc:
        with (
            tc.tile_pool(name="consts", bufs=1) as cpool,
            tc.tile_pool(name="xin", bufs=2) as xpool,
            tc.tile_pool(name="tent", bufs=2) as tpool,
            tc.tile_pool(name="obuf", bufs=2) as opool,
            tc.tile_pool(name="psA", bufs=1, space="PSUM") as psa_pool,
            tc.tile_pool(name="psO", bufs=2, space="PSUM") as pso_pool,
        ):
            mstat_t = cpool.tile([P, P], f16, tag="mstat")
            mv_t = cpool.tile([P, 4 * 64], f16, tag="mv")
            nc.sync.dma_start(out=mstat_t[:], in_=mstat_ext[:])
            nc.sync.dma_start(out=mv_t[:], in_=mv_ext[:])

            x_ts = {}
            o_ts = {}
            front = {}

            def emit_x(s):
                x_t = xpool.tile([99, sup_pts], f16, tag="x", name=f"x_{s}")
                for rb in (0, 32, 64, 96):
                    nc.sync.dma_start(
                        out=x_t[rb : rb + 3, :],
                        in_=x_ext[:, s * sup_pts : (s + 1) * sup_pts],
                    )
                x_ts[s] = x_t

            def emit_front(p):
                """Affines + tents for pair p (chunks 2p, 2p+1)."""
                s = p // pairs_per_super
                x_t = x_ts[s]
                base = (p % pairs_per_super) * PAIR   # offset within super
                psA2 = psa_pool.tile([P, CHUNK], f32, tag="A2", name=f"psA2_{p}")
                T2 = tpool.tile([P, CHUNK], f16, tag="T2", name=f"T2_{p}")
                for par in range(2):
                    rb, cb = (64, 0) if par == 0 else (96, 64)
                    xs = slice(base + par * CHUNK, base + (par + 1) * CHUNK)
                    nc.tensor.matmul(
                        psA2[cb : cb + 64, :],
                        lhsT=mstat_t[rb : rb + 3, cb : cb + 64],
                        rhs=x_t[rb : rb + 3, xs],
                        start=True,
                        stop=True,
                        tile_position=(rb, cb),
                    )
                nc.scalar.activation(T2[:], psA2[:], AF.Abs)
                nc.scalar.activation(T2[:], T2[:], AF.Relu, bias=1.0, scale=-1.0)

                Ts = [T2]
                for par in range(2):
                    xs = slice(base + par * CHUNK, base + (par + 1) * CHUNK)
                    for k in range(2):
                        pa = psa_pool.tile(
                            [P, CHUNK], f32, tag=f"A{k}{par}", name=f"psA{k}_{p}_{par}"
                        )
                        Tk = tpool.tile(
                            [P, CHUNK], f16, tag=f"T{k}{par}", name=f"T{k}_{p}_{par}"
                        )
                        nc.tensor.matmul(
                            pa[:],
                            lhsT=mstat_t[32 * k : 32 * k + 3, :],
                            rhs=x_t[32 * k : 32 * k + 3, xs],
                            start=True,
                            stop=True,
                            tile_position=(32 * k, 0),
                        )
                        nc.vector._custom_dve(tent_op, out=Tk[:], in0=pa[:])
                        Ts.append(Tk)
                front[p] = Ts   # [T2, T0e, T1e, T0o, T1o]

            def emit_mains(p):
                T2, T0e, T1e, T0o, T1o = front.pop(p)
                s = p // pairs_per_super
                o_t = o_ts[s]
                base = (p % pairs_per_super) * PAIR
                for par in range(2):
                    T0, T1 = (T0e, T1e) if par == 0 else (T0o, T1o)
                    t2b = 0 if par == 0 else 64
                    mvc2 = 128 if par == 0 else 192
                    psO = pso_pool.tile([P, GI * 64], f32, tag="O", name=f"psO_{p}_{par}")
                    for g in range(GI):
                        o_sl = psO[:, g * 64 : (g + 1) * 64]
                        # T2-dependent matmul first: T2 is ready earliest
                        nc.tensor.matmul(
                            o_sl,
                            lhsT=T2[t2b : t2b + 64, g * P : (g + 1) * P],
                            rhs=mv_t[t2b : t2b + 64, mvc2 : mvc2 + 64],
                            start=True,
                            stop=False,
                        )
                        nc.tensor.matmul(
                            o_sl,
                            lhsT=T0[:, g * P : (g + 1) * P],
                            rhs=mv_t[:, 0:64],
                            start=False,
                            stop=False,
                        )
                        nc.tensor.matmul(
                            o_sl,
                            lhsT=T1[:, g * P : (g + 1) * P],
                            rhs=mv_t[:, 64:128],
                            start=False,
                            stop=True,
                        )
                    oc = (base + par * CHUNK) // P * 64
                    nc.scalar.copy(o_t[:, oc : oc + GI * 64], psO[:])

            for p in range(n_pairs + 1):
                if p < n_pairs:
                    s = p // pairs_per_super
                    if p % pairs_per_super == 0:
                        emit_x(s)
                        o_ts[s] = opool.tile(
                            [P, super_i * 64], f32, tag="o", name=f"o_{s}"
                        )
                    emit_front(p)
                if p > 0:
                    pp = p - 1
                    emit_mains(pp)
                    if pp % pairs_per_super == pairs_per_super - 1:
                        s = pp // pairs_per_super
                        nc.sync.dma_start(
                            out=out_ext[:, s * super_i : (s + 1) * super_i, :],
                            in_=o_ts.pop(s)[:],
                        )
    nc.finalize()
    return nc


# ----------------------------------------------------------------------------
# Host entry point
# ----------------------------------------------------------------------------
def _proc_order(x_shard):
    """Permute points into the device processing order n' = c*CHUNK + g*128 + q
    (point = q*IP + c*GI + g), then split fp32 x losslessly into an fp16
    (hi, lo) pair for the PE's fp16 datapath. Pure layout/precision prep."""
    ncp = x_shard.shape[0]
    ip = ncp // P
    xp = np.ascontiguousarray(
        x_shard.reshape(P, ip // GI, GI).transpose(1, 2, 0)
    ).reshape(-1)
    xh = xp.astype(np.float16)
    xl = (xp - xh.astype(np.float32)).astype(np.float16)
    ones = np.ones_like(xh)
    return np.stack([xh, xl, ones])


_PROGRAM_CACHE = {}


def kernel(x, storage, resolutions):
    x = np.asarray(x, np.float32).reshape(-1)
    assert x.shape[0] == N_FULL
    mstat, mv = make_tables(storage, resolutions)

    if NCP not in _PROGRAM_CACHE:
        _PROGRAM_CACHE[NCP] = build_program(NCP)
    nc = _PROGRAM_CACHE[NCP]

    in_maps = []
    for c in range(N_CORES):
        shard = x[c * NCP : (c + 1) * NCP]
        in_maps.append({"x": _proc_order(shard), "mstat": mstat, "mv": mv})
    res = run_bass_kernel_spmd(nc, in_maps, list(range(N_CORES)))
    outs = [r["out"].reshape(NCP, LEVELS, FEAT) for r in res.results]
    return np.concatenate(outs, axis=0)
